# revision 4
# baseline (speedup 1.0000x reference)
"""Trainium2 Bass kernel for ChamferLoss (B=8, C=3, N=4096), 8 NeuronCores.

Strategy: data-parallel over batch. Core b computes batch b fully:
  D[n,m] = ||x_n||^2 + ||y_m||^2 - 2 x_n.y_m   (x = ori, y = adv points)
  d1 = mean_n relu(min_m D),  d2 = mean_m relu(min_n D)
Host combines: mean_b max(d1_b, d2_b).

The -2*x.y matmul has contraction K=3; fp32 matmul is 4x slower on PE, so
each fp32 value v is split v = vh + vl (bf16 pair) and the product uses the
3-term expansion  x.y ~= xh.yh + xh.yl + xl.yh  (error ~2^-16 relative).
The squared norms are folded into the same matmul via constant-one rows, so
PSUM holds complete distance values and one reduce-min per row-block gives
the row minima. Column minima come from a second pass with roles swapped.
"""

import sys

sys.path.insert(0, "/opt/trn_rl_repo")

import numpy as np

import concourse.bass as bass  # noqa: F401  (registers engine types)
import concourse.tile as tile
from concourse import bacc, bass_utils, mybir

B, C, N = 8, 3, 4096
NCORES = 8
NO = 32  # n_outer blocks of 128
NI = 128  # n_inner
F32 = mybir.dt.float32
BF16 = mybir.dt.bfloat16
K = 13  # contraction rows: 9 coord product terms + 2 sq rows + 2 one rows

_CACHE = {}


def _prep_pointset(nc, tc, sb, v_dram):
    """Load [3, 4096] fp32 points; return dict of packed SBUF tiles.

    Layouts: vh/vl/m2h/m2l are [96, 128] bf16 (partition = 32*c + n_outer,
    free = n_inner). v2h/v2l are [32, 128] bf16 (partition = n_outer).
    """
    vp = sb.tile([96, 128], F32)
    nc.sync.dma_start(vp[:], v_dram.rearrange("c (no ni) -> (c no) ni", ni=NI))

    vh = sb.tile([96, 128], BF16)
    nc.vector.tensor_copy(vh[:], vp[:])
    vl = sb.tile([96, 128], BF16)
    nc.vector.tensor_sub(vl[:], vp[:], vh[:])
    m2h = sb.tile([96, 128], BF16)
    nc.vector.tensor_scalar_mul(m2h[:], vh[:], -2.0)
    m2l = sb.tile([96, 128], BF16)
    nc.vector.tensor_scalar_mul(m2l[:], vl[:], -2.0)

    vsq = sb.tile([96, 128], F32)
    nc.vector.tensor_mul(vsq[:], vp[:], vp[:])
    # gather the three c-blocks side by side on partitions 0..31
    vsqr = sb.tile([32, 384], F32)
    for c in range(3):
        nc.sync.dma_start(vsqr[:, 128 * c : 128 * (c + 1)], vsq[32 * c : 32 * (c + 1), :])
    v2 = sb.tile([32, 128], F32)
    nc.vector.tensor_add(v2[:], vsqr[:, 0:128], vsqr[:, 128:256])
    nc.vector.tensor_add(v2[:], v2[:], vsqr[:, 256:384])
    v2h = sb.tile([32, 128], BF16)
    nc.vector.tensor_copy(v2h[:], v2[:])
    v2l = sb.tile([32, 128], BF16)
    nc.vector.tensor_sub(v2l[:], v2[:], v2h[:])
    return dict(vh=vh, vl=vl, m2h=m2h, m2l=m2l, v2h=v2h, v2l=v2l)


def _assemble_lhs(nc, sb, p, ones, name):
    """lhsT image [13, 4096] bf16: rows 3c+{0,1}=m2h_c, 3c+2=m2l_c,
    rows 9,10 = ones, rows 11,12 = v2h, v2l."""
    m = sb.tile([K, N], BF16, name=name)
    for c in range(3):
        src_h = p["m2h"][32 * c : 32 * (c + 1), :]
        src_l = p["m2l"][32 * c : 32 * (c + 1), :]
        nc.sync.dma_start(m[3 * c : 3 * c + 1, :], src_h)
        nc.sync.dma_start(m[3 * c + 1 : 3 * c + 2, :], src_h)
        nc.sync.dma_start(m[3 * c + 2 : 3 * c + 3, :], src_l)
    nc.sync.dma_start(m[9:10, :], ones[:])
    nc.sync.dma_start(m[10:11, :], ones[:])
    nc.sync.dma_start(m[11:12, :], p["v2h"][:])
    nc.sync.dma_start(m[12:13, :], p["v2l"][:])
    return m


def _assemble_rhs(nc, sb, p, ones, name):
    """rhs image [13, 4096] bf16: rows 3c+{0,2}=vh_c, 3c+1=vl_c,
    rows 9,10 = v2h, v2l, rows 11,12 = ones."""
    m = sb.tile([K, N], BF16, name=name)
    for c in range(3):
        src_h = p["vh"][32 * c : 32 * (c + 1), :]
        src_l = p["vl"][32 * c : 32 * (c + 1), :]
        nc.sync.dma_start(m[3 * c : 3 * c + 1, :], src_h)
        nc.sync.dma_start(m[3 * c + 2 : 3 * c + 3, :], src_h)
        nc.sync.dma_start(m[3 * c + 1 : 3 * c + 2, :], src_l)
    nc.sync.dma_start(m[9:10, :], p["v2h"][:])
    nc.sync.dma_start(m[10:11, :], p["v2l"][:])
    nc.sync.dma_start(m[11:12, :], ones[:])
    nc.sync.dma_start(m[12:13, :], ones[:])
    return m


def _build():
    nc = bacc.Bacc("TRN2", target_bir_lowering=False, debug=False)
    x_d = nc.dram_tensor("x", [C, N], F32, kind="ExternalInput").ap()
    y_d = nc.dram_tensor("y", [C, N], F32, kind="ExternalInput").ap()
    out_d = nc.dram_tensor("o", [128, 2], F32, kind="ExternalOutput").ap()

    with tile.TileContext(nc) as tc:
        with (
            tc.tile_pool(name="prep", bufs=1) as prep,
            tc.tile_pool(name="mats", bufs=1) as mats,
            tc.tile_pool(name="parts", bufs=1) as parts,
            tc.tile_pool(name="psum", bufs=2, space="PSUM") as psum,
        ):
            px = _prep_pointset(nc, tc, prep, x_d)
            py = _prep_pointset(nc, tc, prep, y_d)
            ones = prep.tile([32, 128], BF16)
            nc.gpsimd.memset(ones[:], 1.0)
            LX = _assemble_lhs(nc, mats, px, ones, "LX")
            RX = _assemble_rhs(nc, mats, px, ones, "RX")
            LY = _assemble_lhs(nc, mats, py, ones, "LY")
            RY = _assemble_rhs(nc, mats, py, ones, "RY")

            partials = []
            for d, (L, R) in enumerate(((LX, RY), (LY, RX))):
                pt = parts.tile([128, 64], F32, name=f"part{d}")
                partials.append(pt)
                for r in range(NO):
                    lhsT = L[:, 128 * r : 128 * (r + 1)]
                    for h in range(2):
                        p = psum.tile([128, 2048], F32, name="pp")
                        for j in range(4):
                            nc.tensor.matmul(
                                p[:, 512 * j : 512 * (j + 1)],
                                lhsT,
                                R[:, 2048 * h + 512 * j : 2048 * h + 512 * (j + 1)],
                                start=True,
                                stop=True,
                            )
                        nc.vector.tensor_reduce(
                            pt[:, 2 * r + h : 2 * r + h + 1],
                            p[:],
                            axis=mybir.AxisListType.X,
                            op=mybir.AluOpType.min,
                        )

            osb = parts.tile([128, 2], F32)
            for d in range(2):
                rm = parts.tile([128, 32], F32, name=f"rm{d}")
                nc.vector.tensor_reduce(
                    rm[:],
                    partials[d][:].rearrange("p (no h) -> p no h", h=2),
                    axis=mybir.AxisListType.X,
                    op=mybir.AluOpType.min,
                )
                nc.vector.tensor_scalar_max(rm[:], rm[:], 0.0)
                nc.vector.reduce_sum(
                    osb[:, d : d + 1], rm[:], axis=mybir.AxisListType.X
                )
            nc.sync.dma_start(out_d[:], osb[:])

    nc.compile()
    return nc


def kernel(ori_pcs: np.ndarray, adv_pcs: np.ndarray) -> np.ndarray:
    if "nc" not in _CACHE:
        _CACHE["nc"] = _build()
    nc = _CACHE["nc"]

    ori = np.ascontiguousarray(np.asarray(ori_pcs, dtype=np.float32))
    adv = np.ascontiguousarray(np.asarray(adv_pcs, dtype=np.float32))
    in_maps = [{"x": ori[b], "y": adv[b]} for b in range(B)]
    res = bass_utils.run_bass_kernel_spmd(nc, in_maps, core_ids=list(range(NCORES)))

    vals = []
    for b in range(B):
        o = res.results[b]["o"].astype(np.float64)
        d1 = o[:, 0].sum() / N
        d2 = o[:, 1].sum() / N
        vals.append(max(d1, d2))
    return np.array(np.mean(vals), dtype=np.float32)
